# revision 29
# baseline (speedup 1.0000x reference)
"""Trainium2 Bass kernel for nn_AnchorPlusLoss (8 NeuronCores, data-parallel).

Math (per batch b):
  x = embedding; x[..., :2] += abs_coords          # coords fold into first 2 dims
  ssq[i,j] = ||x_i - x_j||^2 = r_i + r_j - 2 x_i.x_j   (Gram matrix)
  dist = sqrt(ssq);  sim = sigmoid(5 - dist) = 0.5 + 0.5*tanh((5 - dist)/2)
  L_i = log( sum_{j not masked, j != i} exp(sim_ij) )
  loss = sum_{(i,j) in mask} softplus(L_i - sim_ij)
       = sum_m [ L_i - sim_ij + exp(sim_ij)/exp(L_i) ]  (+O(e^{-2t}), t >= 5)

Sharding: 4 batches x 1024 rows = 4096 rows, 512 rows per core.

Device layout is TRANSPOSED per core: partitions = j (neighbor index, 8 tiles
of 128), free axis = i (the core's 512 anchor rows). ssq comes from bf16
hi/lo-split matmuls (3 accumulating matmuls ~ f32 precision at bf16 speed):
  ssq + EPS = sum_k S[k,j] * Mv[k,i],  S = [x; r; 1], Mv = [-2x; 1; r+EPS]
ScalarE: dist=Sqrt(psum); th=Tanh(-.5d+2.5); E=Exp(.5th+.5)  (2 table sets,
         the sqrt set preloaded by a dummy activation during the matmul phase)
VectorE: pt = m*th, pe = m*E (bf16 tensor_tensor, quarter-tiled)
TensorE: per-i sums over j via ones-vector matmuls into PSUM:
  SE_i = sum_j E, SmE_i = sum_j m*E, Smth_i = sum_j m*th
Host:   negsum = SE - SmE - (1-diag)*E_ii ; L = log(negsum)
        loss_i = npos*L - (0.5*npos + 0.5*Smth) + SmE/negsum ; sum all.
"""

import sys
import time

sys.path.insert(0, "/opt/trn_rl_repo")

import numpy as np
import ml_dtypes

N_CORES = 8
B, N, E = 4, 1024, 32
ROWS = 512          # rows (i) per core = free width
P = 128             # partitions
TJ = N // P         # 8 j-tiles per core
HALF = 4 * ROWS     # 2048: four j-tiles per PSUM half
QW = 2 * ROWS       # 1024: quarter width (two j-tiles)
K = 34              # contraction dim: 32 emb + 2 augmentation rows
EPS = 0.25          # added to ssq so the diagonal never goes negative
SMV_W = 3 * N       # packed operand tensor width: s_hi | s_lo | mv_hi | mv_lo

_nc_cache = {}
_runner_cache = {}


def _build_body(nc, mybir, ext, sb, it, variant="full", strict=False):
    """Emit one iteration of the kernel body (fresh semaphores).

    strict=True adds same-engine RAW waits that CoreSim's race detector
    demands; the in-order engines don't need them on hardware."""
    AF = mybir.ActivationFunctionType
    smv_ext, m_ext, acc_ext = ext
    smv_sb, m_sb, dist_sb, th_sb, e_sb, pt_sb, pe_sb, acc_sb, scr_sb, ps, rps = sb
    ones = nc.const_aps.aps[(mybir.dt.bfloat16, 1.0)]
    # smv layout: [s_hi (N) | mv_hi (ROWS) | s_lo (N) | mv_lo (ROWS)]
    s_hi = smv_sb[:, 0:N]
    mv_hi = smv_sb[:, N : N + ROWS]
    s_lo = smv_sb[:, N + ROWS : 2 * N + ROWS]
    mv_lo = smv_sb[:, 2 * N + ROWS : 2 * N + 2 * ROWS]
    HI_W = N + ROWS

    do_in_dma = variant in ("full", "novec", "mm", "dma")
    do_mm = variant in ("full", "novec", "mm")
    do_scalar = variant in ("full", "novec")
    do_vec = variant == "full"

    masksem = nc.alloc_semaphore(f"masksem{it}")
    smsem = nc.alloc_semaphore(f"smsem{it}")
    losem = nc.alloc_semaphore(f"losem{it}")
    tsem = nc.alloc_semaphore(f"tsem{it}")
    sqsem = nc.alloc_semaphore(f"sqsem{it}")
    thsem = nc.alloc_semaphore(f"thsem{it}")
    esem = nc.alloc_semaphore(f"esem{it}")
    ptsem = nc.alloc_semaphore(f"ptsem{it}")
    pesem = nc.alloc_semaphore(f"pesem{it}")
    redsem = nc.alloc_semaphore(f"redsem{it}")
    csem = nc.alloc_semaphore(f"csem{it}")
    osem = nc.alloc_semaphore(f"osem{it}")

    def mm_hihi(tensor, t, q):
        off = (q % 2) * QW + (t % 2) * ROWS
        sl = ps[:, off : off + ROWS]
        st = slice(t * P, (t + 1) * P)
        tensor.matmul(sl, s_hi[:, st], mv_hi, start=True, stop=False)

    def mm_lo(tensor, t, q):
        off = (q % 2) * QW + (t % 2) * ROWS
        sl = ps[:, off : off + ROWS]
        st = slice(t * P, (t + 1) * P)
        tensor.matmul(sl, s_hi[:, st], mv_lo, start=False, stop=False)
        tensor.matmul(sl, s_lo[:, st], mv_hi, start=False, stop=True).then_inc(
            tsem
        )

    def red_tiles(tensor, q, src, bank, sem=None):
        for t in (2 * q, 2 * q + 1):
            inst = tensor.matmul(
                rps[32 * bank : 32 * bank + 1, :],
                ones,
                src[:, t * ROWS : (t + 1) * ROWS],
                start=(t == 0),
                stop=(t == 7),
                skip_group_check=True,
            )
            if sem is not None and t == 7:
                inst.then_inc(sem)

    with nc.Block() as block:

        @block.sync
        def _(sync):
            if do_in_dma:
                sync.dma_start(smv_sb[:, :HI_W], smv_ext[:, :HI_W]).then_inc(
                    smsem, 16
                )
                sync.dma_start(smv_sb[:, HI_W:], smv_ext[:, HI_W:]).then_inc(
                    losem, 16
                )
            if variant == "full":
                sync.wait_ge(csem, 1)
            elif variant == "novec":
                sync.wait_ge(esem, 4)
            elif variant == "mm":
                sync.wait_ge(tsem, 8)
            elif variant == "dma":
                sync.wait_ge(smsem, 16)
                sync.wait_ge(losem, 16)
                sync.wait_ge(masksem, 16)
            sync.dma_start(acc_ext[:, :], acc_sb[:, :]).then_inc(osem, 16)
            sync.wait_ge(osem, 16)

        @block.gpsimd
        def _(gpsimd):
            gpsimd.wait_ge(osem, 16)

        @block.scalar
        def _(scalar):
            if do_in_dma:
                scalar.dma_start(m_sb[:, :], m_ext[:, :]).then_inc(masksem, 16)
            if do_scalar:
                # dummy: pull the sqrt table set in while matmuls run
                scalar.activation(
                    scr_sb[0:1, 0:1],
                    nc.const_aps.tensor(0.0, (1, 1), mybir.dt.float32),
                    AF.Sqrt,
                )
                for q in range(4):
                    scalar.wait_ge(tsem, 2 * (q + 1))
                    scalar.activation(
                        dist_sb[:, q * QW : (q + 1) * QW],
                        ps[:, (q % 2) * QW : (q % 2 + 1) * QW],
                        AF.Sqrt,
                    ).then_inc(sqsem)
                for q in range(4):
                    sl = slice(q * QW, (q + 1) * QW)
                    if strict:
                        scalar.wait_ge(sqsem, q + 1)
                    scalar.activation(
                        th_sb[:, sl], dist_sb[:, sl], AF.Tanh, bias=2.5, scale=-0.5
                    ).then_inc(thsem)
                    if strict:
                        scalar.wait_ge(thsem, q + 1)
                    scalar.activation(
                        e_sb[:, sl], th_sb[:, sl], AF.Exp, bias=0.5, scale=0.5
                    ).then_inc(esem)

        if do_mm:

            @block.tensor
            def _(tensor):
                tensor.wait_ge(smsem, 16)
                # q0: hi*hi first (lo halves may still be in flight)
                mm_hihi(tensor, 0, 0)
                mm_hihi(tensor, 1, 0)
                tensor.wait_ge(losem, 16)
                mm_lo(tensor, 0, 0)
                mm_lo(tensor, 1, 0)
                for q in range(1, 4):
                    if do_scalar and q >= 2:
                        tensor.wait_ge(sqsem, q - 1)
                    for t in (2 * q, 2 * q + 1):
                        mm_hihi(tensor, t, q)
                        mm_lo(tensor, t, q)
                if do_vec:
                    for q in range(4):
                        tensor.wait_ge(ptsem, q + 1)
                        red_tiles(tensor, q, pt_sb, 2, sem=redsem)
                        tensor.wait_ge(esem, q + 1)
                        red_tiles(tensor, q, e_sb, 0, sem=redsem)
                        tensor.wait_ge(pesem, q + 1)
                        red_tiles(tensor, q, pe_sb, 1, sem=redsem)

        if do_vec:

            @block.vector
            def _(vector):
                vector.memset(rps[:, :], 0)
                vector.wait_ge(masksem, 16)
                for q in range(4):
                    sl = slice(q * QW, (q + 1) * QW)
                    vector.wait_ge(thsem, q + 1)
                    vector.tensor_tensor(
                        out=pt_sb[:, sl],
                        in0=m_sb[:, sl],
                        in1=th_sb[:, sl],
                        op=mybir.AluOpType.mult,
                    ).then_inc(ptsem)
                    vector.wait_ge(esem, q + 1)
                    vector.tensor_tensor(
                        out=pe_sb[:, sl],
                        in0=m_sb[:, sl],
                        in1=e_sb[:, sl],
                        op=mybir.AluOpType.mult,
                    ).then_inc(pesem)
                vector.wait_ge(redsem, 3)
                vector.tensor_scalar_mul(acc_sb[:, :], rps[:, :], 1.0).then_inc(
                    csem
                )


def _build_nc(iters=1, variant="full", strict=False):
    import concourse.bass as bass
    import concourse.mybir as mybir

    nc = bass.Bass()

    # Register const APs for activation biases (framework only pre-registers 0.0/1.0).
    for v in (2.5, 0.5):
        t = nc.alloc_sbuf_tensor(f"const-f32-{v}", [128, 1], mybir.dt.float32)
        nc.gpsimd.memset(t.ap(), v)
        nc.const_aps.aps[(mybir.dt.float32, v)] = t.ap()
    nc.all_engine_barrier()

    f32, bf16 = mybir.dt.float32, mybir.dt.bfloat16
    ext = (
        nc.declare_dram_parameter("smv", [K, SMV_W], bf16, isOutput=False),
        nc.declare_dram_parameter("mask", [P, TJ * ROWS], bf16, isOutput=False),
        nc.declare_dram_parameter("acc", [65, ROWS], f32, isOutput=True),
    )
    sb = (
        nc.alloc_sbuf_tensor("smv_sb", [K, SMV_W], bf16),
        nc.alloc_sbuf_tensor("m_sb", [P, TJ * ROWS], bf16),
        nc.alloc_sbuf_tensor("dist_sb", [P, TJ * ROWS], f32),
        nc.alloc_sbuf_tensor("th_sb", [P, TJ * ROWS], bf16),
        nc.alloc_sbuf_tensor("e_sb", [P, TJ * ROWS], bf16),
        nc.alloc_sbuf_tensor("pt_sb", [P, TJ * ROWS], bf16),
        nc.alloc_sbuf_tensor("pe_sb", [P, TJ * ROWS], bf16),
        nc.alloc_sbuf_tensor("acc_sb", [65, ROWS], f32),
        nc.alloc_sbuf_tensor("scr_sb", [1, 1], f32),
        nc.alloc_psum_tensor("ps", [P, HALF], f32),
        nc.alloc_psum_tensor("rps", [65, ROWS], f32),
    )

    snap = nc._state.snapshot_sems()
    for it in range(iters):
        _build_body(nc, mybir, ext, sb, it, variant=variant, strict=strict)
        if it < iters - 1:
            nc.clear_and_free_semaphores(nc._state.allocated_since(snap))
            nc.all_engine_barrier()
            nc._state.restore_sems(snap)
    return nc


def _get_nc(iters=1, variant="full", strict=False):
    key = (iters, variant, strict)
    if key not in _nc_cache:
        _nc_cache[key] = _build_nc(iters, variant, strict)
    return _nc_cache[key]


def _split_bf16(a):
    hi = a.astype(ml_dtypes.bfloat16)
    lo = (a - hi.astype(np.float64)).astype(ml_dtypes.bfloat16)
    return hi, lo


def _host_prep(embedding, abs_coords, patch_mask):
    """Build per-core input maps."""
    x = embedding.astype(np.float64).copy()  # [B,N,E]
    x[:, :, :2] += abs_coords.astype(np.float64)
    r = np.einsum("bne,bne->bn", x, x)  # [B,N]

    in_maps = []
    for c in range(N_CORES):
        b, i0 = c // 2, 512 * (c % 2)
        xt = x[b].T  # [E, N]
        s = np.empty((K, N), np.float64)
        s[:E] = xt
        s[E] = r[b]
        s[E + 1] = 1.0
        mv = np.empty((K, ROWS), np.float64)
        mv[:E] = -2.0 * xt[:, i0 : i0 + ROWS]
        mv[E] = 1.0
        mv[E + 1] = r[b, i0 : i0 + ROWS] + EPS
        s_hi, s_lo = _split_bf16(s)
        mv_hi, mv_lo = _split_bf16(mv)
        smv = np.concatenate([s_hi, mv_hi, s_lo, mv_lo], axis=1)
        # m_sb[p, 512t + il] = mask[b, i0+il, 128t+p]
        m = (
            patch_mask[b].T[:, i0 : i0 + ROWS]
            .reshape(TJ, P, ROWS)
            .transpose(1, 0, 2)
            .reshape(P, TJ * ROWS)
            .astype(ml_dtypes.bfloat16)
        )
        in_maps.append(
            {"smv": np.ascontiguousarray(smv), "mask": np.ascontiguousarray(m)}
        )
    return in_maps


def _host_combine(results, patch_mask):
    """Per-row logs + final sum on host (4096 rows, trivial)."""
    # device diagonal: dist_ii = sqrt(EPS), E_ii = exp(0.5*tanh((5-d)/2)+0.5)
    d_ii = np.sqrt(EPS)
    e_ii = np.exp(0.5 * np.tanh((5.0 - d_ii) / 2.0) + 0.5)

    total = 0.0
    for c in range(N_CORES):
        b, i0 = c // 2, 512 * (c % 2)
        acc = results[c]["acc"].astype(np.float64)
        se, sme, smth = acc[0], acc[32], acc[64]
        mrows = patch_mask[b, i0 : i0 + ROWS, :].astype(np.float64)
        npos = mrows.sum(axis=1)
        diag = np.diagonal(patch_mask[b], 0)[i0 : i0 + ROWS].astype(np.float64)
        negsum = se - sme - (1.0 - diag) * e_ii
        L = np.log(negsum)
        loss_rows = npos * L - (0.5 * npos + 0.5 * smth) + sme / negsum
        total += loss_rows.sum()
    return total


def _make_runner(nc, in_maps):
    """Persistent jitted SPMD runner mirroring bass2jax.run_bass_via_pjrt.

    Returns f() -> list[dict[name, np.ndarray]]; repeated calls reuse the
    compiled executable so wall-clock deltas reflect device execution.
    """
    import jax
    from jax.sharding import Mesh, PartitionSpec, NamedSharding
    from jax.experimental.shard_map import shard_map
    import concourse.mybir as mybir
    from concourse import bass2jax

    bass2jax.install_neuronx_cc_hook()
    nc.finalize()

    partition_name = nc.partition_id_tensor.name if nc.partition_id_tensor else None
    in_names, out_names, out_avals, zero_outs = [], [], [], []
    for alloc in nc.m.functions[0].allocations:
        if not isinstance(alloc, mybir.MemoryLocationSet):
            continue
        name = alloc.memorylocations[0].name
        if alloc.kind == "ExternalInput":
            if name != partition_name:
                in_names.append(name)
        elif alloc.kind == "ExternalOutput":
            shape = tuple(alloc.tensor_shape)
            dtype = mybir.dt.np(alloc.dtype)
            out_names.append(name)
            out_avals.append(jax.core.ShapedArray(shape, dtype))
            zero_outs.append(np.zeros(shape, dtype))
    n_params = len(in_names)
    n_outs = len(out_avals)
    in_names_all = in_names + out_names
    if partition_name is not None:
        in_names_all.append(partition_name)

    donate = ()

    def _body(*args):
        operands = list(args)
        if partition_name is not None:
            operands.append(bass2jax.partition_id_tensor())
        outs = bass2jax._bass_exec_p.bind(
            *operands,
            out_avals=tuple(out_avals),
            in_names=tuple(in_names_all),
            out_names=tuple(out_names),
            lowering_input_output_aliases=(),
            sim_require_finite=True,
            sim_require_nnan=True,
            nc=nc,
        )
        return tuple(outs)

    devices = jax.devices()[:N_CORES]
    mesh = Mesh(np.asarray(devices), ("core",))
    in_specs = (PartitionSpec("core"),) * (n_params + n_outs)
    out_specs = (PartitionSpec("core"),) * len(out_names)
    sharded = jax.jit(
        shard_map(
            _body, mesh=mesh, in_specs=in_specs, out_specs=out_specs, check_rep=False
        ),
        keep_unused=True,
    )
    per_core = [[np.asarray(m[name]) for name in in_names] for m in in_maps]
    concat_in = [
        np.concatenate([per_core[c][i] for c in range(N_CORES)], axis=0)
        for i in range(n_params)
    ]
    shard = NamedSharding(mesh, PartitionSpec("core"))
    concat_in_dev = [jax.device_put(a, shard) for a in concat_in]

    concat_zeros_dev = [
        jax.device_put(
            np.zeros((N_CORES * z.shape[0], *z.shape[1:]), z.dtype), shard
        )
        for z in zero_outs
    ]

    def run(fetch=True, block=True):
        out_arrs = sharded(*concat_in_dev, *concat_zeros_dev)
        if not fetch:
            if block:
                jax.block_until_ready(out_arrs)
                return None
            return out_arrs
        out_arrs = [np.asarray(a) for a in out_arrs]
        return [
            {
                name: out_arrs[i].reshape(N_CORES, *out_avals[i].shape)[c]
                for i, name in enumerate(out_names)
            }
            for c in range(N_CORES)
        ]

    return run


def _run(embedding, abs_coords, patch_mask, trace=False):
    from concourse.bass_utils import run_bass_kernel_spmd

    nc = _get_nc(1)
    in_maps = _host_prep(embedding, abs_coords, patch_mask)
    res = run_bass_kernel_spmd(
        nc, in_maps, core_ids=list(range(N_CORES)), trace=trace
    )
    total = _host_combine(res.results, patch_mask)
    return np.asarray(total, dtype=np.float32), res


def bench(embedding, abs_coords, patch_mask, iters=257, variant="full"):
    """Measure per-iteration HW time: async-queue k executions of an
    iters-looped NEFF, block once; slope over k cancels dispatch noise."""
    import jax

    in_maps = _host_prep(embedding, abs_coords, patch_mask)
    key = (iters, variant)
    if key not in _runner_cache:
        _runner_cache[key] = _make_runner(_get_nc(iters, variant), in_maps)
    f = _runner_cache[key]
    out = f()  # warm-up + correctness output

    def batch(k):
        outs = None
        t0 = time.perf_counter()
        for _ in range(k):
            outs = f(fetch=False, block=False)
        jax.block_until_ready(outs)
        return time.perf_counter() - t0

    batch(2)
    t2 = min(batch(2) for _ in range(4))
    t10 = min(batch(10) for _ in range(4))
    ns = (t10 - t2) / (8 * iters) * 1e9
    return ns, out


def kernel(embedding, abs_coords, patch_mask):
    out, _ = _run(
        np.asarray(embedding), np.asarray(abs_coords), np.asarray(patch_mask)
    )
    return out
